# revision 46
# baseline (speedup 1.0000x reference)
"""FASTMultiHeadAttention (polynomial softmax + RPE bias, causal) on 8 trn2 cores.

Math per (b,h):   s[i,j] = q_i.k_j + q_i.rpe[n-1+i-j]
                  score  = 1 + s + 0.5 s^2    (= 0.5[(s+1)^2 + 1], 0.5 cancels)
                  o_i    = sum_{j<=i} score v_j / sum_{j<=i} score

Device pipeline per (b,h)  [B*H = 32 units, 4 per core], processed in pairs
of 128-row tiles:
  - m2r[i,u]  = q_i . rpeR[u]          (PE matmul, rpeR = reversed rpe band)
  - bias tile = shear-read of m2r      (SBUF->SBUF DMA, coupled access pattern)
  - psum_s    = qT.T @ kT              (PE)
  - w         = (psum_s + 1) + bias    (DVE scalar_tensor_tensor, evac to SBUF)
  - diag mask via gpsimd.affine_select; zero-fill the causal tail
  - scoreT    = blockwise-transpose(w) (XBAR DMA transpose, no PE involved)
  - scoreT    = scoreT^2               (in-place, ACT/DVE alternating)
  - oT[65,512] += vaug_J.T @ scoreT    (PE, accumulated over J; col 64 = ones)
  - oT evac to SBUF, DMA to HBM.

Host post-processing: add cumsum(v) correction (the a0=1 term) in f64, divide
by the denominator row, transpose back to [n, d].

s and m2r matmuls are K=64 row-packed onto the two 64-row halves of the PE
array (tile_position (0,0)/(64,0)); operands for the upper half live in
SBUF partitions 64-127.
"""

import sys

if "/opt/trn_rl_repo" not in sys.path:
    sys.path.insert(0, "/opt/trn_rl_repo")

import ml_dtypes
import numpy as np

import bass_rust
import concourse.bacc as bacc
import concourse.bass as bass
import concourse.mybir as mybir
import concourse.tile as tile
from concourse.bass_utils import run_bass_kernel_spmd

F32 = mybir.dt.float32
BF16 = mybir.dt.bfloat16

B, H, N, D = 2, 16, 1024, 64
NBH = B * H  # 32
N_CORES = 8
BH_PER_CORE = NBH // N_CORES  # 4
NT = N // 128  # 8 row tiles
SROW = 1280  # m2r row-buffer width per tile (elements)
RPE_W = 1280  # reversed rpe band (1151 valid + zero pad)

# Matmul chunks: exact causal widths, split at 512 (PSUM bank limit). All PE
# matmuls run in bf16 (1 cyc/col, FWL weight loads, K=64 row-packing).
def _chunks(total):
    out = []
    c = 0
    while c < total:
        out.append((c, min(512, total - c)))
        c += 512
    return out


S_CHUNKS = {I: _chunks(128 * (I + 1)) for I in range(8)}
M2R_CHUNKS = {I: _chunks(255 + 128 * I) for I in range(8)}


def _ap(t_ap, pairs, offset=0):
    """Custom access pattern on a tile: pairs = [[step, count], ...] (elements)."""
    cp = t_ap.copy()
    cp.ap = bass_rust.VecI64Pair(pairs)
    cp.offset = offset
    return cp


def build_program():
    nc = bacc.Bacc(
        "TRN2", target_bir_lowering=False, debug=False, num_devices=N_CORES
    )

    qT_d = nc.dram_tensor("qT", [BH_PER_CORE, 64, N], BF16, kind="ExternalInput").ap()
    kT_d = nc.dram_tensor("kT", [BH_PER_CORE, 64, N], BF16, kind="ExternalInput").ap()
    va_d = nc.dram_tensor("va", [BH_PER_CORE, N, 65], BF16, kind="ExternalInput").ap()
    rpe_d = nc.dram_tensor("rpeR", [64, RPE_W], BF16, kind="ExternalInput").ap()
    oT_d = nc.dram_tensor("oT", [BH_PER_CORE, 65, N], F32, kind="ExternalOutput").ap()

    with tile.TileContext(nc) as tc:
        with (
            tc.tile_pool(name="const", bufs=1) as cpool,
            tc.tile_pool(name="io", bufs=4) as io,
            tc.tile_pool(name="m2r", bufs=4) as m2rp,
            tc.tile_pool(name="wrow", bufs=5) as wp,
            tc.tile_pool(name="bias", bufs=4) as bp,
            tc.tile_pool(name="sct", bufs=4) as scp,
            tc.tile_pool(name="fin", bufs=4) as fp,
            tc.tile_pool(name="psms", bufs=3, space="PSUM") as ps_s,
            tc.tile_pool(name="psmr", bufs=3, space="PSUM") as ps_mr,
            tc.tile_pool(name="psot", bufs=2, space="PSUM") as ps_ot,
        ):
            # rpeR duplicated into partitions 64-127 for the upper-half
            # row-packed m2r matmuls.
            rpeR = cpool.tile([128, RPE_W], BF16)
            nc.sync.dma_start(rpeR[64:128, :], rpe_d[:])
            nc.sync.dma_start(rpeR[0:64, :], rpe_d[:])

            evac_flip = [0]  # global ACT/DVE alternation for engine work

            def evac_p1(dst, src):
                # psum -> sbuf evac with +1 folded in (mostly ACT; DVE is
                # saturated by the merges)
                if evac_flip[0] % 4 != 3:
                    nc.scalar.activation(
                        dst, src, mybir.ActivationFunctionType.Copy, bias=1.0
                    )
                else:
                    nc.vector.tensor_scalar_add(dst, src, 1.0)
                evac_flip[0] += 1

            def emit_B(st):
                # oT accumulation over J (512-wide moving) for a finished quad
                m, g, scoreT, va, oT_fin = st
                need = 128 * (4 * g + 4)
                pot = ps_ot.tile([65, 512], F32, tag="ot")
                njs = 4 * g + 4
                for J in range(njs):
                    rhs = _ap(
                        scoreT[:],
                        [[4 * need, 128], [need, 4], [1, 128]],
                        offset=128 * J,
                    )
                    out3 = _ap(pot[:], [[512, 65], [128, 4], [1, 128]])
                    nc.tensor.matmul(
                        out3,
                        va[:, 65 * J : 65 * (J + 1)],
                        rhs,
                        start=(J == 0),
                        stop=(J == njs - 1),
                    )
                # evac raw numerator/denominator rows; host finishes.
                nc.scalar.copy(oT_fin[:, 512 * g : 512 * (g + 1)], pot[:])
                if g == 1:
                    nc.sync.dma_start(oT_d[m], oT_fin[:])

            def emit_io(m):
                qT = io.tile([128, N], BF16, tag="qT")
                if m == 0:
                    nc.gpsimd.dma_start(qT[64:128, 0:256], qT_d[m][:, 0:256])
                    nc.gpsimd.dma_start(qT[64:128, 256:1024], qT_d[m][:, 256:1024])
                else:
                    nc.gpsimd.dma_start(qT[64:128, :], qT_d[m])
                nc.gpsimd.dma_start(qT[0:64, :], qT_d[m])
                kT = io.tile([64, N], BF16, tag="kT")
                if m == 0:
                    nc.scalar.dma_start(kT[:, 0:256], kT_d[m][:, 0:256])
                    nc.scalar.dma_start(kT[:, 256:1024], kT_d[m][:, 256:1024])
                else:
                    nc.gpsimd.dma_start(kT[:], kT_d[m])
                va = io.tile([128, NT * 65], BF16, tag="va")
                nc.gpsimd.dma_start(
                    va[:].rearrange("p (a d) -> p a d", a=NT),
                    va_d[m].rearrange("(a b) d -> b a d", a=NT),
                )
                oT_fin = fp.tile([65, N], F32, tag="ofin")
                return dict(qT=qT, kT=kT, va=va, fin=oT_fin)

            def emit_A1_pair(st, g, pr):
                # m2r band + shear for one pair
                qT = st["qT"]
                need = 128 * (4 * g + 4)
                Iodd = 4 * g + 2 * pr + 1
                band = 255 + 128 * Iodd
                m2r2 = m2rp.tile([128, 2 * SROW], F32, tag="m2r")
                wrow2 = wp.tile([128, 2 * need], BF16, tag="wrow")
                for t2 in range(2):
                    W = 128 * (Iodd + t2)
                    if W < need:
                        nc.vector.memset(
                            wrow2[:, need * t2 + W : need * (t2 + 1)]
                            .bitcast(F32),
                            0.0,
                        )
                for t2 in range(2):
                    I = Iodd - 1 + t2
                    u0 = 896 - 128 * I
                    for c, wd in _chunks(band):
                        pm = ps_mr.tile([128, 512], F32, tag="mr")
                        nc.tensor.matmul(
                            pm[:, :wd],
                            qT[64:128, 128 * I : 128 * (I + 1)],
                            rpeR[64:128, u0 + c : u0 + c + wd],
                            start=True,
                            stop=True,
                            tile_position=(64, 0),
                        )
                        evac_p1(
                            m2r2[:, SROW * t2 + c : SROW * t2 + c + wd],
                            pm[:, :wd],
                        )
                bias2 = bp.tile([128, 2048], F32, tag="bias")
                Wmax = 128 * (Iodd + 1)
                shq = nc.sync if pr == 0 else nc.scalar
                shq.dma_start(
                    _ap(bias2[:], [[2048, 128], [1024, 2], [1, Wmax]]),
                    _ap(
                        m2r2[:],
                        [[2 * SROW - 1, 128], [SROW, 2], [1, Wmax]],
                        offset=127,
                    ),
                )
                return (Iodd, wrow2, bias2)

            def emit_A2_pair(st, g, pair, scoreT, pr):
                # s matmuls, merge, square, mask, xbar for one pair
                qT, kT = st["qT"], st["kT"]
                need = 128 * (4 * g + 4)
                Iodd, wrow2, bias2 = pair
                for t2 in range(2):
                    I = Iodd - 1 + t2
                    W = 128 * (I + 1)
                    for c, wd in S_CHUNKS[I]:
                        psz = ps_s.tile([128, 512], F32, tag="mm")
                        nc.tensor.matmul(
                            psz[:, :wd],
                            qT[0:64, 128 * I : 128 * (I + 1)],
                            kT[:, c : c + wd],
                            start=True,
                            stop=True,
                            tile_position=(0, 0),
                        )
                        nc.vector.tensor_add(
                            wrow2[:, need * t2 + c : need * t2 + c + wd],
                            psz[:, :wd],
                            bias2[:, 1024 * t2 + c : 1024 * t2 + c + wd],
                        )
                    reg = wrow2[:, need * t2 : need * t2 + W]
                    if t2 == 0:
                        nc.scalar.activation(
                            reg, reg, mybir.ActivationFunctionType.Square
                        )
                    else:
                        nc.vector.tensor_mul(reg, reg, reg)
                    nc.gpsimd.affine_select(
                        wrow2[:, need * t2 + W - 128 : need * t2 + W],
                        wrow2[:, need * t2 + W - 128 : need * t2 + W],
                        pattern=[[-1, 128]],
                        compare_op=mybir.AluOpType.is_ge,
                        fill=0.0,
                        base=0,
                        channel_multiplier=1,
                    )
                # blockwise transpose via XBAR DMA (sync queue ONLY;
                # the scalar-queue variant is broken on hardware)
                nc.sync.dma_start_transpose(
                    scoreT[
                        :, 2 * need * pr : 2 * need * (pr + 1)
                    ].rearrange("p (b i) -> p b i", i=128),
                    wrow2[:],
                )

            def emit_B(m, st, g, scoreT):
                need = 128 * (4 * g + 4)
                pot = ps_ot.tile([65, 512], F32, tag="ot")
                njs = 4 * g + 4
                for J in range(njs):
                    rhs = _ap(
                        scoreT[:],
                        [[4 * need, 128], [need, 4], [1, 128]],
                        offset=128 * J,
                    )
                    out3 = _ap(pot[:], [[512, 65], [128, 4], [1, 128]])
                    nc.tensor.matmul(
                        out3,
                        st["va"][:, 65 * J : 65 * (J + 1)],
                        rhs,
                        start=(J == 0),
                        stop=(J == njs - 1),
                    )
                nc.vector.tensor_copy(
                    st["fin"][:, 512 * g : 512 * (g + 1)], pot[:]
                )
                nc.gpsimd.dma_start(
                    oT_d[m][:, 512 * g : 512 * (g + 1)],
                    st["fin"][:, 512 * g : 512 * (g + 1)],
                )

            # two units interleaved at phase granularity: any one chain's
            # latency is hidden by the sibling unit's independent work
            for mp in range(BH_PER_CORE // 2):
                ms = (2 * mp, 2 * mp + 1)
                sts = {m: emit_io(m) for m in ms}
                for g in range(NT // 4):
                    need = 128 * (4 * g + 4)
                    prs = {}
                    scs = {}
                    for m in ms:
                        prs[m] = [
                            emit_A1_pair(sts[m], g, pr) for pr in range(2)
                        ]
                    for m in ms:
                        scoreT = scp.tile([128, 4 * need], BF16, tag="scoreT")
                        scs[m] = scoreT
                        for pr in range(2):
                            emit_A2_pair(sts[m], g, prs[m][pr], scoreT, pr)
                    for m in ms:
                        emit_B(m, sts[m], g, scs[m])

    nc.compile()
    return nc


_NC_CACHE = {}


def get_program():
    if "nc" not in _NC_CACHE:
        _NC_CACHE["nc"] = build_program()
    return _NC_CACHE["nc"]


def prepare_inputs(q, k, v, rpe_matrix):
    """Host-side prep: returns per-core input maps."""
    q = np.asarray(q, dtype=np.float32).reshape(NBH, N, D)
    k = np.asarray(k, dtype=np.float32).reshape(NBH, N, D)
    v = np.asarray(v, dtype=np.float32).reshape(NBH, N, D)
    rpe = np.asarray(rpe_matrix, dtype=np.float32)

    BF = ml_dtypes.bfloat16
    qT = np.ascontiguousarray(q.transpose(0, 2, 1)).astype(BF)  # [32, 64, 1024]
    kT = np.ascontiguousarray(k.transpose(0, 2, 1)).astype(BF)
    va = np.concatenate([v, np.ones((NBH, N, 1), np.float32)], axis=2).astype(
        BF
    )  # [32,1024,65]

    # reversed rpe band: rpeR[:, u] = rpe[2046 - u] for u in [0, 1151)
    rpeR = np.zeros((64, RPE_W), np.float32)
    rpeR[:, :1151] = rpe[2046:895:-1].T
    rpeR = rpeR.astype(BF)

    in_maps = []
    for c in range(N_CORES):
        sl = slice(c * BH_PER_CORE, (c + 1) * BH_PER_CORE)
        in_maps.append(
            {
                "qT": np.ascontiguousarray(qT[sl]),
                "kT": np.ascontiguousarray(kT[sl]),
                "va": np.ascontiguousarray(va[sl]),
                "rpeR": rpeR,
            }
        )
    return in_maps, va


def run(q, k, v, rpe_matrix, trace=False):
    nc = get_program()
    in_maps, va = prepare_inputs(q, k, v, rpe_matrix)
    res = run_bass_kernel_spmd(nc, in_maps, list(range(N_CORES)), trace=trace)
    outs = [res.results[c]["oT"] for c in range(N_CORES)]
    oT = np.concatenate(outs, axis=0).astype(np.float64)  # [32, 65, 1024]
    # host epilogue: add the a0=1 prefix term (cumsum of [v, 1]), divide.
    corr = np.cumsum(va.astype(np.float64), axis=1)  # [32, 1024, 65]
    num = oT[:, :64, :].transpose(0, 2, 1) + corr[:, :, :64]
    den = oT[:, 64, :] + corr[:, :, 64]
    o = (num / den[:, :, None]).astype(np.float32)
    return o.reshape(B, H, N, D), res


def kernel(q, k, v, drop_noise=None, rpe_matrix=None, p=2, **kw):
    o, _ = run(q, k, v, rpe_matrix)
    return o


if __name__ == "__main__":
    rng = np.random.default_rng(0)
    q = rng.standard_normal((B, H, N, D), dtype=np.float32)
    k = rng.standard_normal((B, H, N, D), dtype=np.float32)
    v = rng.standard_normal((B, H, N, D), dtype=np.float32)
    rpe = rng.standard_normal((2 * N - 1, D), dtype=np.float32)
    o, _ = run(q, k, v, rpe)
    print("out", o.shape, o.dtype, np.abs(o).max())


# revision 47
# speedup vs baseline: 1.2211x; 1.2211x over previous
"""FASTMultiHeadAttention (polynomial softmax + RPE bias, causal) on 8 trn2 cores.

Math per (b,h):   s[i,j] = q_i.k_j + q_i.rpe[n-1+i-j]
                  score  = 1 + s + 0.5 s^2    (= 0.5[(s+1)^2 + 1], 0.5 cancels)
                  o_i    = sum_{j<=i} score v_j / sum_{j<=i} score

Device pipeline per (b,h)  [B*H = 32 units, 4 per core], processed in pairs
of 128-row tiles:
  - m2r[i,u]  = q_i . rpeR[u]          (PE matmul, rpeR = reversed rpe band)
  - bias tile = shear-read of m2r      (SBUF->SBUF DMA, coupled access pattern)
  - psum_s    = qT.T @ kT              (PE)
  - w         = (psum_s + 1) + bias    (DVE scalar_tensor_tensor, evac to SBUF)
  - diag mask via gpsimd.affine_select; zero-fill the causal tail
  - scoreT    = blockwise-transpose(w) (XBAR DMA transpose, no PE involved)
  - scoreT    = scoreT^2               (in-place, ACT/DVE alternating)
  - oT[65,512] += vaug_J.T @ scoreT    (PE, accumulated over J; col 64 = ones)
  - oT evac to SBUF, DMA to HBM.

Host post-processing: add cumsum(v) correction (the a0=1 term) in f64, divide
by the denominator row, transpose back to [n, d].

s and m2r matmuls are K=64 row-packed onto the two 64-row halves of the PE
array (tile_position (0,0)/(64,0)); operands for the upper half live in
SBUF partitions 64-127.
"""

import sys

if "/opt/trn_rl_repo" not in sys.path:
    sys.path.insert(0, "/opt/trn_rl_repo")

import ml_dtypes
import numpy as np

import bass_rust
import concourse.bacc as bacc
import concourse.bass as bass
import concourse.mybir as mybir
import concourse.tile as tile
from concourse.bass_utils import run_bass_kernel_spmd

F32 = mybir.dt.float32
BF16 = mybir.dt.bfloat16

B, H, N, D = 2, 16, 1024, 64
NBH = B * H  # 32
N_CORES = 8
BH_PER_CORE = NBH // N_CORES  # 4
NT = N // 128  # 8 row tiles
SROW = 1280  # m2r row-buffer width per tile (elements)
RPE_W = 1280  # reversed rpe band (1151 valid + zero pad)

# Matmul chunks: exact causal widths, split at 512 (PSUM bank limit). All PE
# matmuls run in bf16 (1 cyc/col, FWL weight loads, K=64 row-packing).
def _chunks(total):
    out = []
    c = 0
    while c < total:
        out.append((c, min(512, total - c)))
        c += 512
    return out


S_CHUNKS = {I: _chunks(128 * (I + 1)) for I in range(8)}
M2R_CHUNKS = {I: _chunks(255 + 128 * I) for I in range(8)}


def _ap(t_ap, pairs, offset=0):
    """Custom access pattern on a tile: pairs = [[step, count], ...] (elements)."""
    cp = t_ap.copy()
    cp.ap = bass_rust.VecI64Pair(pairs)
    cp.offset = offset
    return cp


def build_program():
    nc = bacc.Bacc(
        "TRN2", target_bir_lowering=False, debug=False, num_devices=N_CORES
    )

    qT_d = nc.dram_tensor("qT", [BH_PER_CORE, 64, N], BF16, kind="ExternalInput").ap()
    kT_d = nc.dram_tensor("kT", [BH_PER_CORE, 64, N], BF16, kind="ExternalInput").ap()
    va_d = nc.dram_tensor("va", [BH_PER_CORE, N, 65], BF16, kind="ExternalInput").ap()
    rpe_d = nc.dram_tensor("rpeR", [64, RPE_W], BF16, kind="ExternalInput").ap()
    oT_d = nc.dram_tensor("oT", [BH_PER_CORE, 65, N], F32, kind="ExternalOutput").ap()

    with tile.TileContext(nc) as tc:
        with (
            tc.tile_pool(name="const", bufs=1) as cpool,
            tc.tile_pool(name="io", bufs=4) as io,
            tc.tile_pool(name="m2r", bufs=4) as m2rp,
            tc.tile_pool(name="wrow", bufs=5) as wp,
            tc.tile_pool(name="bias", bufs=4) as bp,
            tc.tile_pool(name="sct", bufs=4) as scp,
            tc.tile_pool(name="fin", bufs=4) as fp,
            tc.tile_pool(name="psms", bufs=3, space="PSUM") as ps_s,
            tc.tile_pool(name="psmr", bufs=3, space="PSUM") as ps_mr,
            tc.tile_pool(name="psot", bufs=2, space="PSUM") as ps_ot,
        ):
            # rpeR duplicated into partitions 64-127 for the upper-half
            # row-packed m2r matmuls.
            rpeR = cpool.tile([128, RPE_W], BF16)
            nc.sync.dma_start(rpeR[64:128, :], rpe_d[:])
            nc.sync.dma_start(rpeR[0:64, :], rpe_d[:])

            evac_flip = [0]  # global ACT/DVE alternation for engine work

            def evac_p1(dst, src):
                # psum -> sbuf evac with +1 folded in (mostly ACT; DVE is
                # saturated by the merges)
                if evac_flip[0] % 4 != 3:
                    nc.scalar.activation(
                        dst, src, mybir.ActivationFunctionType.Copy, bias=1.0
                    )
                else:
                    nc.vector.tensor_scalar_add(dst, src, 1.0)
                evac_flip[0] += 1

            def emit_B(st):
                # oT accumulation over J (512-wide moving) for a finished quad
                m, g, scoreT, va, oT_fin = st
                need = 128 * (4 * g + 4)
                pot = ps_ot.tile([65, 512], F32, tag="ot")
                njs = 4 * g + 4
                for J in range(njs):
                    rhs = _ap(
                        scoreT[:],
                        [[4 * need, 128], [need, 4], [1, 128]],
                        offset=128 * J,
                    )
                    out3 = _ap(pot[:], [[512, 65], [128, 4], [1, 128]])
                    nc.tensor.matmul(
                        out3,
                        va[:, 65 * J : 65 * (J + 1)],
                        rhs,
                        start=(J == 0),
                        stop=(J == njs - 1),
                    )
                # evac raw numerator/denominator rows; host finishes.
                nc.scalar.copy(oT_fin[:, 512 * g : 512 * (g + 1)], pot[:])
                if g == 1:
                    nc.sync.dma_start(oT_d[m], oT_fin[:])

            def emit_io(m):
                qT = io.tile([128, N], BF16, tag="qT")
                if m == 0:
                    nc.gpsimd.dma_start(qT[64:128, 0:256], qT_d[m][:, 0:256])
                    nc.gpsimd.dma_start(qT[64:128, 256:1024], qT_d[m][:, 256:1024])
                else:
                    nc.gpsimd.dma_start(qT[64:128, :], qT_d[m])
                nc.gpsimd.dma_start(qT[0:64, :], qT_d[m])
                kT = io.tile([64, N], BF16, tag="kT")
                if m == 0:
                    nc.scalar.dma_start(kT[:, 0:256], kT_d[m][:, 0:256])
                    nc.scalar.dma_start(kT[:, 256:1024], kT_d[m][:, 256:1024])
                else:
                    nc.gpsimd.dma_start(kT[:], kT_d[m])
                va = io.tile([128, NT * 65], BF16, tag="va")
                nc.gpsimd.dma_start(
                    va[:].rearrange("p (a d) -> p a d", a=NT),
                    va_d[m].rearrange("(a b) d -> b a d", a=NT),
                )
                oT_fin = fp.tile([65, N], F32, tag="ofin")
                return dict(qT=qT, kT=kT, va=va, fin=oT_fin)

            def emit_A1_pair(st, g, pr):
                # m2r band + shear for one pair
                qT = st["qT"]
                need = 128 * (4 * g + 4)
                Iodd = 4 * g + 2 * pr + 1
                band = 255 + 128 * Iodd
                m2r2 = m2rp.tile([128, 2 * SROW], F32, tag="m2r")
                wrow2 = wp.tile([128, 2 * need], BF16, tag="wrow")
                for t2 in range(2):
                    W = 128 * (Iodd + t2)
                    if W < need:
                        nc.vector.memset(
                            wrow2[:, need * t2 + W : need * (t2 + 1)]
                            .bitcast(F32),
                            0.0,
                        )
                for t2 in range(2):
                    I = Iodd - 1 + t2
                    u0 = 896 - 128 * I
                    for c, wd in _chunks(band):
                        pm = ps_mr.tile([128, 512], F32, tag="mr")
                        nc.tensor.matmul(
                            pm[:, :wd],
                            qT[64:128, 128 * I : 128 * (I + 1)],
                            rpeR[64:128, u0 + c : u0 + c + wd],
                            start=True,
                            stop=True,
                            tile_position=(64, 0),
                        )
                        evac_p1(
                            m2r2[:, SROW * t2 + c : SROW * t2 + c + wd],
                            pm[:, :wd],
                        )
                bias2 = bp.tile([128, 2048], F32, tag="bias")
                Wmax = 128 * (Iodd + 1)
                nc.sync.dma_start(
                    _ap(bias2[:], [[2048, 128], [1024, 2], [1, Wmax]]),
                    _ap(
                        m2r2[:],
                        [[2 * SROW - 1, 128], [SROW, 2], [1, Wmax]],
                        offset=127,
                    ),
                )
                return (Iodd, wrow2, bias2)

            def emit_A2_pair(st, g, pair, scoreT, pr):
                # s matmuls, merge, square, mask, xbar for one pair
                qT, kT = st["qT"], st["kT"]
                need = 128 * (4 * g + 4)
                Iodd, wrow2, bias2 = pair
                for t2 in range(2):
                    I = Iodd - 1 + t2
                    W = 128 * (I + 1)
                    for c, wd in S_CHUNKS[I]:
                        psz = ps_s.tile([128, 512], F32, tag="mm")
                        nc.tensor.matmul(
                            psz[:, :wd],
                            qT[0:64, 128 * I : 128 * (I + 1)],
                            kT[:, c : c + wd],
                            start=True,
                            stop=True,
                            tile_position=(0, 0),
                        )
                        nc.vector.tensor_add(
                            wrow2[:, need * t2 + c : need * t2 + c + wd],
                            psz[:, :wd],
                            bias2[:, 1024 * t2 + c : 1024 * t2 + c + wd],
                        )
                    reg = wrow2[:, need * t2 : need * t2 + W]
                    if t2 == 0:
                        nc.scalar.activation(
                            reg, reg, mybir.ActivationFunctionType.Square
                        )
                    else:
                        nc.vector.tensor_mul(reg, reg, reg)
                    nc.gpsimd.affine_select(
                        wrow2[:, need * t2 + W - 128 : need * t2 + W],
                        wrow2[:, need * t2 + W - 128 : need * t2 + W],
                        pattern=[[-1, 128]],
                        compare_op=mybir.AluOpType.is_ge,
                        fill=0.0,
                        base=0,
                        channel_multiplier=1,
                    )
                # blockwise transpose via XBAR DMA (sync queue ONLY;
                # the scalar-queue variant is broken on hardware)
                nc.sync.dma_start_transpose(
                    scoreT[
                        :, 2 * need * pr : 2 * need * (pr + 1)
                    ].rearrange("p (b i) -> p b i", i=128),
                    wrow2[:],
                )

            def emit_B(m, st, g, scoreT):
                need = 128 * (4 * g + 4)
                pot = ps_ot.tile([65, 512], F32, tag="ot")
                njs = 4 * g + 4
                for J in range(njs):
                    rhs = _ap(
                        scoreT[:],
                        [[4 * need, 128], [need, 4], [1, 128]],
                        offset=128 * J,
                    )
                    out3 = _ap(pot[:], [[512, 65], [128, 4], [1, 128]])
                    nc.tensor.matmul(
                        out3,
                        st["va"][:, 65 * J : 65 * (J + 1)],
                        rhs,
                        start=(J == 0),
                        stop=(J == njs - 1),
                    )
                nc.vector.tensor_copy(
                    st["fin"][:, 512 * g : 512 * (g + 1)], pot[:]
                )
                nc.sync.dma_start(
                    oT_d[m][:, 512 * g : 512 * (g + 1)],
                    st["fin"][:, 512 * g : 512 * (g + 1)],
                )

            # two units interleaved at phase granularity: any one chain's
            # latency is hidden by the sibling unit's independent work
            for mp in range(BH_PER_CORE // 2):
                ms = (2 * mp, 2 * mp + 1)
                sts = {m: emit_io(m) for m in ms}
                for g in range(NT // 4):
                    need = 128 * (4 * g + 4)
                    prs = {}
                    scs = {}
                    for m in ms:
                        prs[m] = [
                            emit_A1_pair(sts[m], g, pr) for pr in range(2)
                        ]
                    for m in ms:
                        scoreT = scp.tile([128, 4 * need], BF16, tag="scoreT")
                        scs[m] = scoreT
                        for pr in range(2):
                            emit_A2_pair(sts[m], g, prs[m][pr], scoreT, pr)
                    for m in ms:
                        emit_B(m, sts[m], g, scs[m])

    nc.compile()
    return nc


_NC_CACHE = {}


def get_program():
    if "nc" not in _NC_CACHE:
        _NC_CACHE["nc"] = build_program()
    return _NC_CACHE["nc"]


def prepare_inputs(q, k, v, rpe_matrix):
    """Host-side prep: returns per-core input maps."""
    q = np.asarray(q, dtype=np.float32).reshape(NBH, N, D)
    k = np.asarray(k, dtype=np.float32).reshape(NBH, N, D)
    v = np.asarray(v, dtype=np.float32).reshape(NBH, N, D)
    rpe = np.asarray(rpe_matrix, dtype=np.float32)

    BF = ml_dtypes.bfloat16
    qT = np.ascontiguousarray(q.transpose(0, 2, 1)).astype(BF)  # [32, 64, 1024]
    kT = np.ascontiguousarray(k.transpose(0, 2, 1)).astype(BF)
    va = np.concatenate([v, np.ones((NBH, N, 1), np.float32)], axis=2).astype(
        BF
    )  # [32,1024,65]

    # reversed rpe band: rpeR[:, u] = rpe[2046 - u] for u in [0, 1151)
    rpeR = np.zeros((64, RPE_W), np.float32)
    rpeR[:, :1151] = rpe[2046:895:-1].T
    rpeR = rpeR.astype(BF)

    in_maps = []
    for c in range(N_CORES):
        sl = slice(c * BH_PER_CORE, (c + 1) * BH_PER_CORE)
        in_maps.append(
            {
                "qT": np.ascontiguousarray(qT[sl]),
                "kT": np.ascontiguousarray(kT[sl]),
                "va": np.ascontiguousarray(va[sl]),
                "rpeR": rpeR,
            }
        )
    return in_maps, va


def run(q, k, v, rpe_matrix, trace=False):
    nc = get_program()
    in_maps, va = prepare_inputs(q, k, v, rpe_matrix)
    res = run_bass_kernel_spmd(nc, in_maps, list(range(N_CORES)), trace=trace)
    outs = [res.results[c]["oT"] for c in range(N_CORES)]
    oT = np.concatenate(outs, axis=0).astype(np.float64)  # [32, 65, 1024]
    # host epilogue: add the a0=1 prefix term (cumsum of [v, 1]), divide.
    corr = np.cumsum(va.astype(np.float64), axis=1)  # [32, 1024, 65]
    num = oT[:, :64, :].transpose(0, 2, 1) + corr[:, :, :64]
    den = oT[:, 64, :] + corr[:, :, 64]
    o = (num / den[:, :, None]).astype(np.float32)
    return o.reshape(B, H, N, D), res


def kernel(q, k, v, drop_noise=None, rpe_matrix=None, p=2, **kw):
    o, _ = run(q, k, v, rpe_matrix)
    return o


if __name__ == "__main__":
    rng = np.random.default_rng(0)
    q = rng.standard_normal((B, H, N, D), dtype=np.float32)
    k = rng.standard_normal((B, H, N, D), dtype=np.float32)
    v = rng.standard_normal((B, H, N, D), dtype=np.float32)
    rpe = rng.standard_normal((2 * N - 1, D), dtype=np.float32)
    o, _ = run(q, k, v, rpe)
    print("out", o.shape, o.dtype, np.abs(o).max())
